# revision 19
# baseline (speedup 1.0000x reference)
"""Distributed Trainium2 (Bass/Tile) kernel for a Qwen3-style attention layer.

Full layer: QKV proj -> per-head RMSNorm (q,k) -> RoPE -> GQA SDPA -> o_proj.

Sharding over 8 NeuronCores:
  - tensor-parallel across heads for QKV+attention: core c owns q-heads
    [4c, 4c+4) and kv-head c; hidden_states replicated.
  - AllToAll exchanges attention context so each core ends with all 4096
    context dims for a 256-token slice; o_proj is then token-parallel with
    Wo replicated (streamed). Output: per-core [256, 4096] chunks that the
    host concatenates. No all-reduce needed.

Compute layout: everything lives transposed ([dim, token]) so the PE array
contracts over the partition axis with N=512 moving tiles in bf16.

Software pipeline: the whole kernel is emitted as fine-grained units on a
single schedule. Each batch's attention (scores -> exp -> context ->
softmax-normalize -> ship) interleaves into the FOLLOWING projection
blocks, so the scalar-engine exp stream hides under projection matmuls
and the context AllToAlls fire as early as possible. Batch 1's attention
tail runs in phase 2, interleaved with the first o_proj weight-group
loads. PSUM: proj 2 banks + scores 2 + context 2 + aux 2 = 8.

DMA queues: nc.sync (SP HWDGE) for proj loads / ships / Wo grp0 / cx1 /
output stores; nc.scalar (ACT HWDGE) for constants + Wo grps 1-3;
nc.gpsimd (SWDGE) for collective triggers + cx0. A tiny AllToAll at t~0
absorbs cross-core launch stagger. The rope half-swap is a permutation
matmul (no SBUF-SBUF DMA), pipelined one block behind the projection.
"""

import numpy as np
import ml_dtypes

import concourse.bass as bass
import concourse.mybir as mybir
from concourse import bacc
from concourse.tile import TileContext
from concourse.bass_utils import run_bass_kernel_spmd
from concourse.masks import make_identity

F32 = mybir.dt.float32
BF16 = mybir.dt.bfloat16
BF16_NP = ml_dtypes.bfloat16

N_CORES = 8

FULL_CFG = dict(B=2, S=1024, HID=4096, H=32, KV=8, D=128, eps=1e-6)


def build_program(B=2, S=1024, HID=4096, H=32, KV=8, D=128, eps=1e-6):
    cores = N_CORES
    assert D == 128 and H % cores == 0 and KV == cores and B == 2
    HQ = H // cores            # q heads per core
    HH = HQ // 2               # heads per a2a half (last batch)
    T = B * S                  # total tokens
    HCH = HID // 128           # hidden-dim chunks of 128
    TT = min(512, S)           # projection token tile (within batch)
    TPB = S // TT              # projection tiles per batch
    KB = S // 128              # key blocks per batch
    QT = min(512, S)           # attention q tile
    QTB = S // QT              # q tiles per batch
    TC = T // cores            # output tokens per core
    TCB = TC // B              # per-batch token slice per core
    ICH = (H * D) // 128       # o_proj contraction chunks (32)
    OH = min(512, HID // 2)    # o_proj hid tile width
    NHG = HID // OH            # number of hid groups
    GSZ = 2                    # hid groups per o_proj block
    assert NHG % GSZ == 0
    scale = float(D) ** -0.5
    MULT = mybir.AluOpType.mult
    SW = QTB * QT              # full q row per batch (== S)

    nc = bacc.Bacc("TRN2", target_bir_lowering=False, debug=False,
                   num_devices=cores)

    hT = nc.dram_tensor("hT", [B, HCH, 128, S], BF16, kind="ExternalInput")
    wq = nc.dram_tensor("wq", [HQ, 128, HCH * 128], BF16, kind="ExternalInput")
    wk = nc.dram_tensor("wk", [128, HCH * 128], BF16, kind="ExternalInput")
    wv = nc.dram_tensor("wv", [128, HCH * 128], BF16, kind="ExternalInput")
    wo = nc.dram_tensor("wo", [ICH, 128, HID], BF16, kind="ExternalInput")
    cosT = nc.dram_tensor("cosT", [128, S], BF16, kind="ExternalInput")
    csinT = nc.dram_tensor("csinT", [128, S], BF16, kind="ExternalInput")
    qw = nc.dram_tensor("qw", [128, 1], F32, kind="ExternalInput")
    kw = nc.dram_tensor("kw", [128, 1], F32, kind="ExternalInput")
    out = nc.dram_tensor("out", [TC, HID], F32, kind="ExternalOutput")

    with TileContext(nc) as tc:
        with (
            tc.tile_pool(name="const", bufs=1) as cp,
            tc.tile_pool(name="dram", bufs=1, space="DRAM") as dramp,
            tc.tile_pool(name="qkv", bufs=1) as p_qkv,
            tc.tile_pool(name="work", bufs=2) as p_work,
            tc.tile_pool(name="pt", bufs=2) as p_pt,
            tc.tile_pool(name="psum", bufs=1, space="PSUM") as ps_all,
        ):
            ones_s = cp.tile([128, 128], BF16)
            nc.vector.memset(ones_s[:, :], 1.0)
            ident = cp.tile([128, 128], BF16)
            make_identity(nc, ident[:, :])
            # permutation matrix for the rope half-swap (rotate by 64):
            # perm[i, j] = 1 iff j == (i+64) % 128  (self-inverse)
            perm_s = cp.tile([128, 128], BF16)
            nc.vector.memset(perm_s[:, :], 0.0)
            make_identity(nc, perm_s[0:64, 64:128], nomemset=True)
            make_identity(nc, perm_s[64:128, 0:64], nomemset=True)
            eps_s = cp.tile([128, 1], F32)
            nc.vector.memset(eps_s[:, :], eps)
            cos_s = cp.tile([128, S], BF16)
            nc.scalar.dma_start(out=cos_s[:, :], in_=cosT[:, :])
            csin_s = cp.tile([128, S], BF16)
            nc.scalar.dma_start(out=csin_s[:, :], in_=csinT[:, :])
            qw_s = cp.tile([128, 1], F32)
            nc.scalar.dma_start(out=qw_s[:, :], in_=qw[:, :])
            kw_s = cp.tile([128, 1], F32)
            nc.scalar.dma_start(out=kw_s[:, :], in_=kw[:, :])

            # Tiny sync collective: absorbs the per-core launch stagger while
            # proj0 computes, so the real collectives find the cores aligned.
            sync_in = dramp.tile([cores, 64], BF16, name="synci")
            sync_out = dramp.tile([cores, 64], BF16, name="synco")
            nc.gpsimd.collective_compute(
                "AllToAll", mybir.AluOpType.bypass,
                replica_groups=[list(range(cores))],
                ins=[sync_in.opt()], outs=[sync_out.opt()])

            # A2A buffers: [(dst_core*128 + p), (local_head*TCB + t)] so the
            # received block from src core j sits at rows [j*128, (j+1)*128)
            # with 1KB-contiguous rows.
            a2a0_in = dramp.tile([cores * 128, HQ * TCB], BF16, name="a2a0i")
            a2a0_out = dramp.tile([cores * 128, HQ * TCB], BF16, name="a2a0o")
            a2a1_in = [dramp.tile([cores * 128, HH * TCB], BF16,
                                  tag=f"a2a1i{p}", name=f"a2a1i{p}")
                       for p in range(2)]
            a2a1_out = [dramp.tile([cores * 128, HH * TCB], BF16,
                                   tag=f"a2a1o{p}", name=f"a2a1o{p}")
                        for p in range(2)]

            qT_s = p_qkv.tile([128, HQ * T], BF16, tag="qT")
            kT_s = p_qkv.tile([128, T], BF16, tag="kT")
            vnat_s = p_qkv.tile([128, T], BF16, tag="vnat")
            ctxT_s = p_qkv.tile([128, HQ * T], BF16, tag="ctxT")

            # ---------------- rope finish pipeline (one block behind) ------
            pending = []

            def flush_pending():
                for qn_t, dst, pos in pending:
                    qswp = ps_all.tile([128, TT], F32, tag="aux",
                                       name="qswp", bufs=2)
                    nc.tensor.matmul(qswp[:, :], lhsT=perm_s[:, :],
                                     rhs=qn_t[:, :], start=True, stop=True)
                    t1 = p_work.tile([128, TT], F32, tag="t1")
                    nc.vector.tensor_mul(t1[:, :], qn_t[:, :],
                                         cos_s[:, pos: pos + TT])
                    t2 = p_work.tile([128, TT], BF16, tag="t2")
                    nc.vector.tensor_mul(t2[:, :], qswp[:, :],
                                         csin_s[:, pos: pos + TT])
                    nc.vector.tensor_add(dst, t1[:, :], t2[:, :])
                pending.clear()

            # ---------------- unit-stream builders -------------------------
            def wload(ob, p_w, chunked=False):
                w_t = p_w.tile([128, HCH * 128], BF16, tag="w", name="w")
                srcw = (wq[ob] if ob < HQ else
                        (wk[:, :] if ob == HQ else wv[:, :]))
                if chunked:
                    for wc in range(4):
                        cs = wc * (HCH // 4) * 128
                        ce = (wc + 1) * (HCH // 4) * 128
                        nc.sync.dma_start(out=w_t[:, cs:ce],
                                          in_=srcw[:, cs:ce])
                else:
                    nc.sync.dma_start(out=w_t[:, :], in_=srcw)
                return w_t

            def block_gen(b, ob, hch, p_w, w_pre=None):
                """Projection output block. Unit 0 emits the weight load
                (so the driver can prime it one block ahead); then 8 matmul
                groups of 8 K-chunks + 2 norm/rope (or v-transpose) chain
                units."""
                w_t = w_pre if w_pre is not None else wload(ob, p_w)
                yield
                for tt in range(TPB):
                    ps = ps_all.tile([128, TT], F32, tag="pj", name="ps",
                                     bufs=2)
                    for cg in range(4):
                        for ch in range(cg * 8, cg * 8 + 8):
                            nc.tensor.matmul(
                                ps[:, :],
                                lhsT=w_t[:, ch * 128:(ch + 1) * 128],
                                rhs=hch[ch][:, tt * TT:(tt + 1) * TT],
                                start=(ch == 0), stop=(ch == HCH - 1))
                        yield
                    if tt == 0:
                        flush_pending()
                    psv = ps[:, :]
                    tg = b * S + tt * TT
                    pos = tt * TT
                    if ob <= HQ:
                        is_q = ob < HQ
                        dst = (qT_s[:, ob * T + tg: ob * T + tg + TT]
                               if is_q else kT_s[:, tg: tg + TT])
                        wcol = qw_s if is_q else kw_s
                        sq = p_work.tile([128, TT], BF16, tag="sq")
                        nc.scalar.square(sq[:, :], psv)
                        ssq = ps_all.tile([128, TT], F32, tag="aux",
                                          name="ssq", bufs=2)
                        nc.tensor.matmul(ssq[:, :], lhsT=ones_s[:, :],
                                         rhs=sq[:, :], start=True, stop=True)
                        # 1/sqrt(ms+eps) as exp(-0.5*ln(.)): ln and exp share
                        # one activation-table set, so the merged pipeline
                        # never pays Exp<->Sqrt table reloads.
                        lg = p_work.tile([128, TT], F32, tag="std")
                        nc.scalar.activation(
                            lg[:, :], ssq[:, :],
                            mybir.ActivationFunctionType.Ln,
                            bias=eps_s[:, :], scale=1.0 / D)
                        rs = p_work.tile([128, TT], F32, tag="rs")
                        nc.scalar.activation(
                            rs[:, :], lg[:, :],
                            mybir.ActivationFunctionType.Exp,
                            scale=-0.5)
                        qn = p_work.tile([128, TT], BF16, tag="qn", bufs=4)
                        nc.vector.scalar_tensor_tensor(
                            qn[:, :], in0=psv, scalar=wcol[:, :],
                            in1=rs[:, :], op0=MULT, op1=MULT)
                        pending.append((qn, dst, pos))
                    else:
                        vt = p_work.tile([128, TT], BF16, tag="vt")
                        nc.scalar.copy(vt[:, :], psv)
                        for tb in range(TT // 128):
                            vtr = ps_all.tile([128, 128], BF16, tag="aux",
                                              name="vtr", bufs=2)
                            nc.tensor.transpose(
                                vtr[:, :], vt[:, tb * 128:(tb + 1) * 128],
                                ident[:, :])
                            tbg = tg // 128 + tb
                            nc.scalar.copy(
                                vnat_s[:, tbg * 128:(tbg + 1) * 128],
                                vtr[:, :])
                    yield

            def make_attn(b):
                """Per-head unit streams: sc (8 score+exp units), cx (8
                context-accumulate units), fin (normalize + ship)."""
                last = b == B - 1
                heads = []
                for h in range(HQ):
                    st = {}
                    qoff = h * T + b * S

                    def sc_gen(st=st, qoff=qoff, b=b):
                        st['pt'] = p_pt.tile([128, KB * SW], BF16, tag="pT",
                                             name="pT")
                        for kb in range(KB):
                            sps = ps_all.tile([128, SW], F32, tag="sps",
                                              name="sps", bufs=1)
                            for qt in range(QTB):
                                nc.tensor.matmul(
                                    sps[:, qt * QT:(qt + 1) * QT],
                                    lhsT=kT_s[:, b * S + kb * 128:
                                              b * S + (kb + 1) * 128],
                                    rhs=qT_s[:, qoff + qt * QT:
                                             qoff + (qt + 1) * QT],
                                    start=True, stop=True)
                            nc.scalar.activation(
                                st['pt'][:, kb * SW:(kb + 1) * SW],
                                sps[:, :],
                                mybir.ActivationFunctionType.Exp,
                                scale=scale)
                            yield

                    def cx_gen(st=st, b=b):
                        st['ctxs'] = [ps_all.tile([128, QT], F32, tag="ctx",
                                                  name="ctx", bufs=2)
                                      for _ in range(QTB)]
                        for kb in range(KB):
                            tbg = (b * S) // 128 + kb
                            for qt in range(QTB):
                                nc.tensor.matmul(
                                    st['ctxs'][qt][:, :],
                                    lhsT=vnat_s[:, tbg * 128:(tbg + 1) * 128],
                                    rhs=st['pt'][:, kb * SW + qt * QT:
                                                 kb * SW + (qt + 1) * QT],
                                    start=(kb == 0), stop=(kb == KB - 1))
                            yield

                    def fin(st=st, qoff=qoff, h=h, last=last, b=b):
                        pt_t = st['pt']
                        denp = p_work.tile([128, SW], BF16, tag="denp")
                        dent = p_work.tile([128, SW], BF16, tag="dent")
                        nc.vector.tensor_add(denp[:, :], pt_t[:, 0:SW],
                                             pt_t[:, SW:2 * SW])
                        nc.vector.tensor_add(dent[:, :],
                                             pt_t[:, 2 * SW:3 * SW],
                                             pt_t[:, 3 * SW:4 * SW])
                        nc.vector.tensor_add(denp[:, :], denp[:, :],
                                             dent[:, :])
                        for g in range(1, KB // 4):
                            nc.vector.tensor_add(
                                dent[:, :],
                                pt_t[:, 4 * g * SW:(4 * g + 1) * SW],
                                pt_t[:, (4 * g + 1) * SW:(4 * g + 2) * SW])
                            nc.vector.tensor_add(denp[:, :], denp[:, :],
                                                 dent[:, :])
                            nc.vector.tensor_add(
                                dent[:, :],
                                pt_t[:, (4 * g + 2) * SW:(4 * g + 3) * SW],
                                pt_t[:, (4 * g + 3) * SW:(4 * g + 4) * SW])
                            nc.vector.tensor_add(denp[:, :], denp[:, :],
                                                 dent[:, :])
                        jpq = cores // QTB
                        pi, hh = h // HH, h % HH
                        for qt in range(QTB):
                            dps = ps_all.tile([128, QT], F32, tag="aux",
                                              name="dps", bufs=2)
                            nc.tensor.matmul(
                                dps[:, :], lhsT=ones_s[:, :],
                                rhs=denp[:, qt * QT:(qt + 1) * QT],
                                start=True, stop=True)
                            rec = p_work.tile([128, QT], F32, tag="rec")
                            nc.vector.reciprocal_approx_fast(rec[:, :],
                                                             dps[:, :])
                            nc.vector.tensor_mul(
                                ctxT_s[:, qoff + qt * QT:
                                       qoff + (qt + 1) * QT],
                                st['ctxs'][qt][:, :], rec[:, :])
                            for j in range(qt * jpq, (qt + 1) * jpq):
                                src = ctxT_s[:, qoff + j * TCB:
                                             qoff + (j + 1) * TCB]
                                if not last:
                                    nc.sync.dma_start(
                                        out=a2a0_in[j * 128:(j + 1) * 128,
                                                    h * TCB:(h + 1) * TCB],
                                        in_=src)
                                else:
                                    nc.sync.dma_start(
                                        out=a2a1_in[pi][
                                            j * 128:(j + 1) * 128,
                                            hh * TCB:(hh + 1) * TCB],
                                        in_=src)
                        if last and hh == HH - 1:
                            nc.gpsimd.collective_compute(
                                "AllToAll", mybir.AluOpType.bypass,
                                replica_groups=[list(range(cores))],
                                ins=[a2a1_in[pi].opt()],
                                outs=[a2a1_out[pi].opt()])
                        if (not last) and h == HQ - 1:
                            nc.gpsimd.collective_compute(
                                "AllToAll", mybir.AluOpType.bypass,
                                replica_groups=[list(range(cores))],
                                ins=[a2a0_in.opt()],
                                outs=[a2a0_out.opt()])

                    heads.append({'sc': sc_gen(), 'cx': cx_gen(), 'fin': fin})
                return heads

            def onesh(fn):
                """Wrap a plain closure as a 1-unit generator."""
                fn()
                yield

            def run_units(*streams):
                """streams: (generator, units_per_turn). Round-robin
                emission until all are exhausted."""
                live = [[g, r] for g, r in streams]
                while live:
                    for s in list(live):
                        g, r = s
                        try:
                            for _ in range(r):
                                next(g)
                        except StopIteration:
                            live.remove(s)

            def chain(*gens):
                for g in gens:
                    yield from g

            def drain(g):
                for _ in g:
                    pass

            # ---------------- phase 1: both batches, pipelined -------------
            with (
                tc.tile_pool(name="hid", bufs=HCH) as p_hid,
                tc.tile_pool(name="wts", bufs=2) as p_w,
            ):
                ob_order = [HQ, HQ + 1, 0, 1, 2, 3]   # k, v, q0..q3

                def load_hch(b, interleave_w):
                    """Issue the hidden-chunk DMAs; for batch 0 the first
                    two weight loads are interleaved so the very first
                    matmuls are not queued behind 8.4MB of hidden state."""
                    hch = []
                    wk_t = wv_t = None
                    for ch in range(HCH):
                        if interleave_w and ch == 0:
                            wk_t = wload(HQ, p_w, chunked=True)
                        t_ = p_hid.tile([128, S], BF16, tag="hid",
                                        name="hid")
                        nc.sync.dma_start(out=t_[:, :], in_=hT[b, ch, :, :])
                        hch.append(t_)
                        if interleave_w and ch == 2:
                            wv_t = wload(HQ + 1, p_w, chunked=True)
                    return hch, wk_t, wv_t

                sc_ = nc.enter_named_scope("b0", True)[0]
                hch0, wk_t, wv_t = load_hch(0, True)
                A0 = make_attn(0)
                pre = {HQ: wk_t, HQ + 1: wv_t}
                g0 = {ob: block_gen(0, ob, hch0, p_w, w_pre=pre.get(ob))
                      for ob in ob_order}
                next(g0[HQ])                # U0 (weights preloaded)
                next(g0[HQ + 1])
                next(g0[0])                 # prefetch wq0
                # k and v interleave so the PE consumes each hidden chunk
                # twice as it arrives (the initial load is DMA-paced).
                run_units((g0[HQ], 1), (g0[HQ + 1], 1))
                next(g0[1])                 # prefetch wq1
                drain(g0[0])                # q0
                next(g0[2])                 # prefetch wq2
                drain(g0[1])                # q1
                next(g0[3])                 # prefetch wq3
                run_units((g0[2], 1), (A0[0]['sc'], 1))
                run_units((g0[3], 1),
                          (chain(A0[0]['cx'], A0[1]['sc']), 2))
                nc.leave_named_scope("b0", sc_, True)

                sc_ = nc.enter_named_scope("b1", True)[0]
                hch1, _, _ = load_hch(1, False)
                A1 = make_attn(1)
                g1 = {ob: block_gen(1, ob, hch1, p_w) for ob in ob_order}
                next(g1[HQ])                # prefetch b1 k,v weights
                next(g1[HQ + 1])
                # batch-0 attention tail rides batch-1's first blocks
                run_units((g1[HQ], 1),
                          (chain(onesh(A0[0]['fin']), A0[1]['cx'],
                                 A0[2]['sc']), 2))
                next(g1[0])
                run_units((g1[HQ + 1], 1),
                          (chain(onesh(A0[1]['fin']), A0[2]['cx'],
                                 A0[3]['sc']), 2))
                next(g1[1])
                run_units((g1[0], 1),
                          (chain(onesh(A0[2]['fin']), A0[3]['cx'],
                                 onesh(A0[3]['fin'])), 1))
                next(g1[2])
                drain(g1[1])
                next(g1[3])
                run_units((g1[2], 1), (A1[0]['sc'], 1))
                run_units((g1[3], 1),
                          (chain(A1[0]['cx'], A1[1]['sc']), 2))
                nc.leave_named_scope("b1", sc_, True)

            # ---------------- phase 2: b1 attn tail + o_proj ---------------
            with (
                tc.tile_pool(name="wo", bufs=1) as p_wo,
                tc.tile_pool(name="cx", bufs=1) as p_cx,
                tc.tile_pool(name="oo", bufs=6) as p_oo,
            ):
                WOB = ICH + 2

                def load_wo_grp(hgs, ics=None, dma=None):
                    wts = {}
                    if ics is None:
                        ics = range(ICH)
                    if dma is None:
                        dma = nc.scalar.dma_start
                    for ic in ics:
                        wo_t = p_wo.tile([128, GSZ * OH], BF16, tag="wo",
                                         name="wo", bufs=WOB)
                        dma(out=wo_t[:, :],
                            in_=wo[ic, :, hgs[0] * OH:(hgs[0] + GSZ) * OH])
                        for i, hg in enumerate(hgs):
                            wts[(hg, ic)] = wo_t[:, i * OH:(i + 1) * OH]
                    return wts

                # cx0 load can go early (a2a0 long done): one SWDGE DMA,
                # 1KB-contiguous per (partition, src core).
                cx_s = [p_cx.tile([128, ICH * TCB], BF16, tag=f"cx{b}",
                                  name=f"cx{b}") for b in range(B)]
                nc.gpsimd.dma_start(
                    out=cx_s[0][:, :].rearrange("p (j f) -> p j f", j=cores),
                    in_=a2a0_out[:, :].rearrange("(j p) f -> p j f", p=128))

                grp0_hgs = [i for i in range(GSZ)]
                grp0_wts = {}

                def wo_gen(ics):
                    def fn():
                        grp0_wts.update(load_wo_grp(
                            grp0_hgs, ics=ics, dma=nc.sync.dma_start))
                    return onesh(fn)

                sc_ = nc.enter_named_scope("tail", True)[0]
                flush_pending()          # finish b1-q3's rope
                A1[0]['fin']()
                run_units((chain(A1[1]['cx'], A1[2]['sc']), 2),
                          (wo_gen(range(0, 11)), 1))
                A1[1]['fin']()           # -> half0 AllToAll
                run_units((chain(A1[2]['cx'], A1[3]['sc']), 2),
                          (wo_gen(range(11, 22)), 1))
                A1[2]['fin']()
                run_units((A1[3]['cx'], 2), (wo_gen(range(22, 32)), 1))
                A1[3]['fin']()           # -> half1 AllToAll
                nc.leave_named_scope("tail", sc_, True)

                sc_ = nc.enter_named_scope("oproj", True)[0]
                # cx1 from the two half-collectives. On the sync queue (NOT
                # gpsimd): a load that waits on a collective would stall the
                # Pool FIFO and delay the other half's trigger.
                cxv = cx_s[1][:, :].rearrange("p (j h t) -> p j h t",
                                              j=cores, t=TCB)
                for pi in range(2):
                    nc.sync.dma_start(
                        out=cxv[:, :, pi * HH:(pi + 1) * HH, :],
                        in_=a2a1_out[pi][:, :].rearrange(
                            "(j p) (hh t) -> p j hh t", p=128, t=TCB))

                # batch 1 contracts the half0-delivered chunks first so its
                # o_proj can start before the half1 collective lands.
                ic_b1 = ([ic for ic in range(ICH) if ic % HQ < HH] +
                         [ic for ic in range(ICH) if ic % HQ >= HH])
                for grp in range(NHG // GSZ):
                    hgs = [grp * GSZ + i for i in range(GSZ)]
                    wts = grp0_wts if grp == 0 else load_wo_grp(hgs)
                    for b in range(B):
                        ics = ic_b1 if b == 1 else list(range(ICH))
                        psos = [ps_all.tile([TCB, OH], F32, tag="aux",
                                            name="pso", bufs=2)
                                for _ in range(GSZ)]
                        for i, ic in enumerate(ics):
                            lw = cx_s[b][:, ic * TCB:(ic + 1) * TCB]
                            for gi in range(GSZ):
                                nc.tensor.matmul(
                                    psos[gi][:, :],
                                    lhsT=lw,
                                    rhs=wts[(hgs[gi], ic)][:, :],
                                    start=(i == 0), stop=(i == ICH - 1))
                        for gi, hg in enumerate(hgs):
                            ot = p_oo.tile([TCB, OH], F32, tag="oout",
                                           name="oout")
                            nc.vector.tensor_copy(ot[:, :], psos[gi][:, :])
                            nc.sync.dma_start(
                                out=out[b * TCB:(b + 1) * TCB,
                                        hg * OH:(hg + 1) * OH],
                                in_=ot[:, :])
                nc.leave_named_scope("oproj", sc_, True)

    _pin_act_tables(nc.m.arch)
    nc.compile()
    return nc


def _pin_act_tables(arch):
    """Constrain the act-table chooser so Exp and Ln both resolve to the
    one set that contains them together (natural_log_exp_and_others).
    Otherwise the pass alternates exp_and_others <-> natural_log on every
    RMSNorm/softmax interleave, paying a ~1.3us table reload each time.
    Only the chooser's view is narrowed; the loaded set id and its table
    contents are unchanged, so results are bit-identical."""
    from concourse.hw_specs import get_activation_tables
    tabs = get_activation_tables(arch)
    both = [name for name, s in tabs.items()
            if any(f.name == 'Exp' for f in s)
            and any(f.name == 'Ln' for f in s)]
    if not both:
        return
    for name, s in tabs.items():
        if name not in both:
            for f in list(s):
                if f.name in ('Exp', 'Ln'):
                    s.discard(f)


def host_prep(inputs, B=2, S=1024, HID=4096, H=32, KV=8, D=128, eps=1e-6):
    """Shard + lay out the full inputs into per-core in_maps."""
    cores = N_CORES
    HQ = H // cores
    T = B * S
    HCH = HID // 128
    ICH = (H * D) // 128

    hs = np.ascontiguousarray(inputs["hidden_states"], dtype=np.float32)
    fc = np.asarray(inputs["freqs_cis"], dtype=np.float32)
    Wq = np.asarray(inputs["Wq"], dtype=np.float32)
    Wk = np.asarray(inputs["Wk"], dtype=np.float32)
    Wv = np.asarray(inputs["Wv"], dtype=np.float32)
    Wo = np.asarray(inputs["Wo"], dtype=np.float32)
    qnw = np.asarray(inputs["q_norm_w"], dtype=np.float32)
    knw = np.asarray(inputs["k_norm_w"], dtype=np.float32)

    # hidden^T chunks: hT[b, ch, p, s] = hs[b, s, ch*128+p]
    hT = np.ascontiguousarray(
        hs.transpose(0, 2, 1).reshape(B, HCH, 128, S)).astype(BF16_NP)

    cos, sin, nsin = fc[0], fc[1], fc[2]      # [S, D]
    cosT = np.ascontiguousarray(cos.T).astype(BF16_NP)    # [128, S]
    csinT = np.concatenate([nsin.T[0:64], sin.T[64:128]], axis=0)
    csinT = np.ascontiguousarray(csinT).astype(BF16_NP)
    qw_col = np.ascontiguousarray(qnw.reshape(128, 1))
    kw_col = np.ascontiguousarray(knw.reshape(128, 1))

    # Wo^T chunks: wo[ic, p, hid] = Wo[hid, ic*128+p]
    woT = np.ascontiguousarray(Wo.T.reshape(ICH, 128, HID)).astype(BF16_NP)

    def prep_w(Wm, nblocks):
        # [nblocks, p, ch*128] with w[ob, p, ch*128+j] = Wm[ob*128+j, ch*128+p]
        a = Wm.reshape(nblocks, 128, HCH, 128).transpose(0, 3, 2, 1)
        return np.ascontiguousarray(a.reshape(nblocks, 128, HCH * 128)) \
            .astype(BF16_NP)

    in_maps = []
    for c in range(cores):
        Wq_c = Wq[c * HQ * D:(c + 1) * HQ * D]
        Wk_c = Wk[c * D:(c + 1) * D]
        Wv_c = Wv[c * D:(c + 1) * D]
        in_maps.append({
            "hT": hT,
            "wq": prep_w(Wq_c, HQ),
            "wk": prep_w(Wk_c, 1)[0],
            "wv": prep_w(Wv_c, 1)[0],
            "wo": woT,
            "cosT": cosT,
            "csinT": csinT,
            "qw": qw_col,
            "kw": kw_col,
        })
    return in_maps


def gather_output(results, B=2, S=1024, HID=4096, **_):
    cores = N_CORES
    TCB = (B * S) // cores // B
    out = np.empty((B, S, HID), dtype=np.float32)
    for c in range(cores):
        o = results[c]["out"]
        for b in range(B):
            out[b, c * TCB:(c + 1) * TCB] = o[b * TCB:(b + 1) * TCB]
    return out


_NC_CACHE = {}


def kernel(**inputs) -> np.ndarray:
    cfg = FULL_CFG
    key = tuple(sorted(cfg.items()))
    if key not in _NC_CACHE:
        _NC_CACHE[key] = build_program(**cfg)
    nc = _NC_CACHE[key]
    in_maps = host_prep(inputs, **cfg)
    res = run_bass_kernel_spmd(nc, in_maps, core_ids=list(range(N_CORES)))
    return gather_output(res.results, **cfg)


# revision 24
# speedup vs baseline: 1.0582x; 1.0582x over previous
"""Distributed Trainium2 (Bass/Tile) kernel for a Qwen3-style attention layer.

Full layer: QKV proj -> per-head RMSNorm (q,k) -> RoPE -> GQA SDPA -> o_proj.

Sharding over 8 NeuronCores:
  - tensor-parallel across heads for QKV+attention: core c owns q-heads
    [4c, 4c+4) and kv-head c; hidden_states replicated.
  - AllToAll exchanges attention context so each core ends with all 4096
    context dims for a 256-token slice; o_proj is then token-parallel with
    Wo replicated (streamed). Output: per-core [256, 4096] chunks that the
    host concatenates. No all-reduce needed.

Compute layout: everything lives transposed ([dim, token]) so the PE array
contracts over the partition axis with N=512 moving tiles in bf16.

Software pipeline: the whole kernel is emitted as fine-grained units on a
single schedule. Each batch's attention (scores -> exp -> context ->
softmax-normalize -> ship) interleaves into the FOLLOWING projection
blocks, so the scalar-engine exp stream hides under projection matmuls
and the context AllToAlls fire as early as possible. Batch 1's attention
tail runs in phase 2, interleaved with the first o_proj weight-group
loads. PSUM: proj 2 banks + scores 2 + context 2 + aux 2 = 8.

DMA queues: nc.sync (SP HWDGE) for proj loads / ships / Wo grp0 / cx1 /
output stores; nc.scalar (ACT HWDGE) for constants + Wo grps 1-3;
nc.gpsimd (SWDGE) for collective triggers + cx0. A tiny AllToAll at t~0
absorbs cross-core launch stagger. The rope half-swap is a permutation
matmul (no SBUF-SBUF DMA), pipelined one block behind the projection.
"""

import numpy as np
import ml_dtypes

import concourse.bass as bass
import concourse.mybir as mybir
from concourse import bacc
from concourse.tile import TileContext
from concourse.bass_utils import run_bass_kernel_spmd
from concourse.masks import make_identity

F32 = mybir.dt.float32
BF16 = mybir.dt.bfloat16
BF16_NP = ml_dtypes.bfloat16

N_CORES = 8

FULL_CFG = dict(B=2, S=1024, HID=4096, H=32, KV=8, D=128, eps=1e-6)


def build_program(B=2, S=1024, HID=4096, H=32, KV=8, D=128, eps=1e-6):
    cores = N_CORES
    assert D == 128 and H % cores == 0 and KV == cores and B == 2
    HQ = H // cores            # q heads per core
    HH = HQ // 2               # heads per a2a half (last batch)
    T = B * S                  # total tokens
    HCH = HID // 128           # hidden-dim chunks of 128
    TT = min(512, S)           # projection token tile (within batch)
    TPB = S // TT              # projection tiles per batch
    KB = S // 128              # key blocks per batch
    QT = min(512, S)           # attention q tile
    QTB = S // QT              # q tiles per batch
    TC = T // cores            # output tokens per core
    TCB = TC // B              # per-batch token slice per core
    ICH = (H * D) // 128       # o_proj contraction chunks (32)
    OH = min(512, HID // 2)    # o_proj hid tile width
    NHG = HID // OH            # number of hid groups
    GSZ = 2                    # hid groups per o_proj block
    assert NHG % GSZ == 0
    scale = float(D) ** -0.5
    MULT = mybir.AluOpType.mult
    SW = QTB * QT              # full q row per batch (== S)

    nc = bacc.Bacc("TRN2", target_bir_lowering=False, debug=False,
                   num_devices=cores)

    hT = nc.dram_tensor("hT", [B, HCH, 128, S], BF16, kind="ExternalInput")
    wq = nc.dram_tensor("wq", [HQ, 128, HCH * 128], BF16, kind="ExternalInput")
    wk = nc.dram_tensor("wk", [128, HCH * 128], BF16, kind="ExternalInput")
    wv = nc.dram_tensor("wv", [128, HCH * 128], BF16, kind="ExternalInput")
    wo = nc.dram_tensor("wo", [ICH, 128, HID], BF16, kind="ExternalInput")
    cosT = nc.dram_tensor("cosT", [128, S], BF16, kind="ExternalInput")
    csinT = nc.dram_tensor("csinT", [128, S], BF16, kind="ExternalInput")
    qw = nc.dram_tensor("qw", [128, 1], F32, kind="ExternalInput")
    kw = nc.dram_tensor("kw", [128, 1], F32, kind="ExternalInput")
    out = nc.dram_tensor("out", [TC, HID], F32, kind="ExternalOutput")

    with TileContext(nc) as tc:
        with (
            tc.tile_pool(name="const", bufs=1) as cp,
            tc.tile_pool(name="dram", bufs=1, space="DRAM") as dramp,
            tc.tile_pool(name="qkv", bufs=1) as p_qkv,
            tc.tile_pool(name="work", bufs=2) as p_work,
            tc.tile_pool(name="pt", bufs=2) as p_pt,
            tc.tile_pool(name="psum", bufs=1, space="PSUM") as ps_all,
        ):
            ones_s = cp.tile([128, 128], BF16)
            nc.vector.memset(ones_s[:, :], 1.0)
            ident = cp.tile([128, 128], BF16)
            make_identity(nc, ident[:, :])
            eps_s = cp.tile([128, 1], F32)
            nc.vector.memset(eps_s[:, :], eps)
            cos_s = cp.tile([128, S], BF16)
            nc.scalar.dma_start(out=cos_s[:, :], in_=cosT[:, :])
            csin_s = cp.tile([128, S], BF16)
            nc.scalar.dma_start(out=csin_s[:, :], in_=csinT[:, :])
            qw_s = cp.tile([128, 1], F32)
            nc.scalar.dma_start(out=qw_s[:, :], in_=qw[:, :])
            kw_s = cp.tile([128, 1], F32)
            nc.scalar.dma_start(out=kw_s[:, :], in_=kw[:, :])

            # Tiny sync collective: absorbs the per-core launch stagger while
            # proj0 computes, so the real collectives find the cores aligned.
            sync_in = dramp.tile([cores, 64], BF16, name="synci")
            sync_out = dramp.tile([cores, 64], BF16, name="synco")
            nc.gpsimd.collective_compute(
                "AllToAll", mybir.AluOpType.bypass,
                replica_groups=[list(range(cores))],
                ins=[sync_in.opt()], outs=[sync_out.opt()])

            # A2A buffers: [(dst_core*128 + p), (local_head*TCB + t)] so the
            # received block from src core j sits at rows [j*128, (j+1)*128)
            # with 1KB-contiguous rows.
            a2a0_in = dramp.tile([cores * 128, HQ * TCB], BF16, name="a2a0i")
            a2a0_out = dramp.tile([cores * 128, HQ * TCB], BF16, name="a2a0o")
            a2a1_in = [dramp.tile([cores * 128, HH * TCB], BF16,
                                  tag=f"a2a1i{p}", name=f"a2a1i{p}")
                       for p in range(2)]
            a2a1_out = [dramp.tile([cores * 128, HH * TCB], BF16,
                                   tag=f"a2a1o{p}", name=f"a2a1o{p}")
                        for p in range(2)]

            qT_s = p_qkv.tile([128, HQ * T], BF16, tag="qT")
            kT_s = p_qkv.tile([128, T], BF16, tag="kT")
            vnat_s = p_qkv.tile([128, T], BF16, tag="vnat")
            ctxT_s = p_qkv.tile([128, HQ * T], BF16, tag="ctxT")

            # ---------------- rope finish pipeline (one block behind) ------
            pending = []

            def flush_pending():
                # half-swap via SBUF-SBUF DMA on the ACT HWDGE queue: keeps
                # the PE free, and by flush time (one block behind) qn is
                # long ready so the dma never stalls the ACT FIFO.
                for qn_t, dst, pos in pending:
                    qsw = p_work.tile([128, TT], BF16, tag="qsw")
                    nc.scalar.dma_start(out=qsw[0:64, :], in_=qn_t[64:128, :])
                    nc.scalar.dma_start(out=qsw[64:128, :], in_=qn_t[0:64, :])
                    t1 = p_work.tile([128, TT], F32, tag="t1")
                    nc.vector.tensor_mul(t1[:, :], qn_t[:, :],
                                         cos_s[:, pos: pos + TT])
                    t2 = p_work.tile([128, TT], BF16, tag="t2")
                    nc.vector.tensor_mul(t2[:, :], qsw[:, :],
                                         csin_s[:, pos: pos + TT])
                    nc.vector.tensor_add(dst, t1[:, :], t2[:, :])
                pending.clear()

            # ---------------- unit-stream builders -------------------------
            def wload(ob, p_w, chunked=False):
                w_t = p_w.tile([128, HCH * 128], BF16, tag="w", name="w")
                srcw = (wq[ob] if ob < HQ else
                        (wk[:, :] if ob == HQ else wv[:, :]))
                if chunked:
                    for wc in range(4):
                        cs = wc * (HCH // 4) * 128
                        ce = (wc + 1) * (HCH // 4) * 128
                        nc.sync.dma_start(out=w_t[:, cs:ce],
                                          in_=srcw[:, cs:ce])
                else:
                    nc.sync.dma_start(out=w_t[:, :], in_=srcw)
                return w_t

            def block_gen(b, ob, hch, p_w, w_pre=None, fine=False):
                """Projection output block. Unit 0 emits the weight load
                (so the driver can prime it one block ahead); then matmul
                groups over the 32 K-chunks + 2 norm/rope (or v-transpose)
                chain units. fine=True splits the first groups into 2-chunk
                units so the first matmuls start as soon as the first
                hidden chunks land (kernel startup is DMA-paced)."""
                w_t = w_pre if w_pre is not None else wload(ob, p_w)
                yield
                for tt in range(TPB):
                    ps = ps_all.tile([128, TT], F32, tag="pj", name="ps",
                                     bufs=2)
                    if fine and tt == 0:
                        groups = [range(i, i + 2) for i in range(0, 8, 2)] \
                            + [range(i, i + 8) for i in range(8, 32, 8)]
                    else:
                        groups = [range(i, i + 8) for i in range(0, 32, 8)]
                    for chs in groups:
                        for ch in chs:
                            nc.tensor.matmul(
                                ps[:, :],
                                lhsT=w_t[:, ch * 128:(ch + 1) * 128],
                                rhs=hch[ch][:, tt * TT:(tt + 1) * TT],
                                start=(ch == 0), stop=(ch == HCH - 1))
                        yield
                    if tt == 0:
                        flush_pending()
                    psv = ps[:, :]
                    tg = b * S + tt * TT
                    pos = tt * TT
                    if ob <= HQ:
                        is_q = ob < HQ
                        dst = (qT_s[:, ob * T + tg: ob * T + tg + TT]
                               if is_q else kT_s[:, tg: tg + TT])
                        wcol = qw_s if is_q else kw_s
                        sq = p_work.tile([128, TT], BF16, tag="sq")
                        nc.scalar.square(sq[:, :], psv)
                        ssq = ps_all.tile([128, TT], F32, tag="aux",
                                          name="ssq", bufs=2)
                        nc.tensor.matmul(ssq[:, :], lhsT=ones_s[:, :],
                                         rhs=sq[:, :], start=True, stop=True)
                        # 1/sqrt(ms+eps) as exp(-0.5*ln(.)): ln and exp share
                        # one activation-table set, so the merged pipeline
                        # never pays Exp<->Sqrt table reloads.
                        lg = p_work.tile([128, TT], F32, tag="std")
                        nc.scalar.activation(
                            lg[:, :], ssq[:, :],
                            mybir.ActivationFunctionType.Ln,
                            bias=eps_s[:, :], scale=1.0 / D)
                        rs = p_work.tile([128, TT], F32, tag="rs")
                        nc.scalar.activation(
                            rs[:, :], lg[:, :],
                            mybir.ActivationFunctionType.Exp,
                            scale=-0.5)
                        qn = p_work.tile([128, TT], BF16, tag="qn", bufs=4)
                        nc.vector.scalar_tensor_tensor(
                            qn[:, :], in0=psv, scalar=wcol[:, :],
                            in1=rs[:, :], op0=MULT, op1=MULT)
                        pending.append((qn, dst, pos))
                    else:
                        vt = p_work.tile([128, TT], BF16, tag="vt")
                        nc.scalar.copy(vt[:, :], psv)
                        for tb in range(TT // 128):
                            vtr = ps_all.tile([128, 128], BF16, tag="aux",
                                              name="vtr", bufs=2)
                            nc.tensor.transpose(
                                vtr[:, :], vt[:, tb * 128:(tb + 1) * 128],
                                ident[:, :])
                            tbg = tg // 128 + tb
                            nc.scalar.copy(
                                vnat_s[:, tbg * 128:(tbg + 1) * 128],
                                vtr[:, :])
                    yield

            def make_attn(b):
                """Per-head unit streams: sc (8 score+exp units), cx (8
                context-accumulate units), fin (normalize + ship)."""
                last = b == B - 1
                heads = []
                for h in range(HQ):
                    st = {}
                    qoff = h * T + b * S

                    def sc_gen(st=st, qoff=qoff, b=b):
                        st['pt'] = p_pt.tile([128, KB * SW], BF16, tag="pT",
                                             name="pT")
                        for kb in range(KB):
                            sps = ps_all.tile([128, SW], F32, tag="sps",
                                              name="sps", bufs=1)
                            for qt in range(QTB):
                                nc.tensor.matmul(
                                    sps[:, qt * QT:(qt + 1) * QT],
                                    lhsT=kT_s[:, b * S + kb * 128:
                                              b * S + (kb + 1) * 128],
                                    rhs=qT_s[:, qoff + qt * QT:
                                             qoff + (qt + 1) * QT],
                                    start=True, stop=True)
                            nc.scalar.activation(
                                st['pt'][:, kb * SW:(kb + 1) * SW],
                                sps[:, :],
                                mybir.ActivationFunctionType.Exp,
                                scale=scale)
                            yield

                    def cx_gen(st=st, b=b):
                        st['ctxs'] = [ps_all.tile([128, QT], F32, tag="ctx",
                                                  name="ctx", bufs=2)
                                      for _ in range(QTB)]
                        for kb in range(KB):
                            tbg = (b * S) // 128 + kb
                            for qt in range(QTB):
                                nc.tensor.matmul(
                                    st['ctxs'][qt][:, :],
                                    lhsT=vnat_s[:, tbg * 128:(tbg + 1) * 128],
                                    rhs=st['pt'][:, kb * SW + qt * QT:
                                                 kb * SW + (qt + 1) * QT],
                                    start=(kb == 0), stop=(kb == KB - 1))
                            yield

                    def fin(st=st, qoff=qoff, h=h, last=last, b=b):
                        pt_t = st['pt']
                        denp = p_work.tile([128, SW], BF16, tag="denp")
                        dent = p_work.tile([128, SW], BF16, tag="dent")
                        nc.vector.tensor_add(denp[:, :], pt_t[:, 0:SW],
                                             pt_t[:, SW:2 * SW])
                        nc.vector.tensor_add(dent[:, :],
                                             pt_t[:, 2 * SW:3 * SW],
                                             pt_t[:, 3 * SW:4 * SW])
                        nc.vector.tensor_add(denp[:, :], denp[:, :],
                                             dent[:, :])
                        for g in range(1, KB // 4):
                            nc.vector.tensor_add(
                                dent[:, :],
                                pt_t[:, 4 * g * SW:(4 * g + 1) * SW],
                                pt_t[:, (4 * g + 1) * SW:(4 * g + 2) * SW])
                            nc.vector.tensor_add(denp[:, :], denp[:, :],
                                                 dent[:, :])
                            nc.vector.tensor_add(
                                dent[:, :],
                                pt_t[:, (4 * g + 2) * SW:(4 * g + 3) * SW],
                                pt_t[:, (4 * g + 3) * SW:(4 * g + 4) * SW])
                            nc.vector.tensor_add(denp[:, :], denp[:, :],
                                                 dent[:, :])
                        jpq = cores // QTB
                        pi, hh = h // HH, h % HH
                        for qt in range(QTB):
                            dps = ps_all.tile([128, QT], F32, tag="aux",
                                              name="dps", bufs=2)
                            nc.tensor.matmul(
                                dps[:, :], lhsT=ones_s[:, :],
                                rhs=denp[:, qt * QT:(qt + 1) * QT],
                                start=True, stop=True)
                            rec = p_work.tile([128, QT], F32, tag="rec")
                            nc.vector.reciprocal_approx_fast(rec[:, :],
                                                             dps[:, :])
                            nc.vector.tensor_mul(
                                ctxT_s[:, qoff + qt * QT:
                                       qoff + (qt + 1) * QT],
                                st['ctxs'][qt][:, :], rec[:, :])
                            for j in range(qt * jpq, (qt + 1) * jpq):
                                src = ctxT_s[:, qoff + j * TCB:
                                             qoff + (j + 1) * TCB]
                                if not last:
                                    nc.sync.dma_start(
                                        out=a2a0_in[j * 128:(j + 1) * 128,
                                                    h * TCB:(h + 1) * TCB],
                                        in_=src)
                                else:
                                    nc.sync.dma_start(
                                        out=a2a1_in[pi][
                                            j * 128:(j + 1) * 128,
                                            hh * TCB:(hh + 1) * TCB],
                                        in_=src)
                        if last and hh == HH - 1:
                            nc.gpsimd.collective_compute(
                                "AllToAll", mybir.AluOpType.bypass,
                                replica_groups=[list(range(cores))],
                                ins=[a2a1_in[pi].opt()],
                                outs=[a2a1_out[pi].opt()])
                        if (not last) and h == HQ - 1:
                            nc.gpsimd.collective_compute(
                                "AllToAll", mybir.AluOpType.bypass,
                                replica_groups=[list(range(cores))],
                                ins=[a2a0_in.opt()],
                                outs=[a2a0_out.opt()])

                    heads.append({'sc': sc_gen(), 'cx': cx_gen(), 'fin': fin})
                return heads

            def onesh(fn):
                """Wrap a plain closure as a 1-unit generator."""
                fn()
                yield

            def run_units(*streams):
                """streams: (generator, units_per_turn). Round-robin
                emission until all are exhausted."""
                live = [[g, r] for g, r in streams]
                while live:
                    for s in list(live):
                        g, r = s
                        try:
                            for _ in range(r):
                                next(g)
                        except StopIteration:
                            live.remove(s)

            def chain(*gens):
                for g in gens:
                    yield from g

            def drain(g):
                for _ in g:
                    pass

            # ---------------- phase 1: both batches, pipelined -------------
            with (
                tc.tile_pool(name="hid", bufs=HCH) as p_hid,
                tc.tile_pool(name="wts", bufs=2) as p_w,
            ):
                ob_order = [HQ, HQ + 1, 0, 1, 2, 3]   # k, v, q0..q3

                def load_hch(b, interleave_w):
                    """Issue the hidden-chunk DMAs; for batch 0 the first
                    two weight loads are interleaved so the very first
                    matmuls are not queued behind 8.4MB of hidden state."""
                    hch = []
                    wk_t = wv_t = None
                    for ch in range(HCH):
                        if interleave_w and ch == 0:
                            wk_t = wload(HQ, p_w, chunked=True)
                        t_ = p_hid.tile([128, S], BF16, tag="hid",
                                        name="hid")
                        nc.sync.dma_start(out=t_[:, :], in_=hT[b, ch, :, :])
                        hch.append(t_)
                        if interleave_w and ch == 2:
                            wv_t = wload(HQ + 1, p_w, chunked=True)
                    return hch, wk_t, wv_t

                sc_ = nc.enter_named_scope("b0", True)[0]
                hch0, wk_t, wv_t = load_hch(0, True)
                A0 = make_attn(0)
                pre = {HQ: wk_t, HQ + 1: wv_t}
                g0 = {ob: block_gen(0, ob, hch0, p_w, w_pre=pre.get(ob),
                                    fine=(ob >= HQ))
                      for ob in ob_order}
                next(g0[HQ])                # U0 (weights preloaded)
                next(g0[HQ + 1])
                next(g0[0])                 # prefetch wq0
                # k and v interleave so the PE consumes each hidden chunk
                # twice as it arrives (the initial load is DMA-paced).
                run_units((g0[HQ], 1), (g0[HQ + 1], 1))
                next(g0[1])                 # prefetch wq1
                drain(g0[0])                # q0
                next(g0[2])                 # prefetch wq2
                drain(g0[1])                # q1
                next(g0[3])                 # prefetch wq3
                run_units((g0[2], 1), (A0[0]['sc'], 1))
                run_units((g0[3], 1),
                          (chain(A0[0]['cx'], A0[1]['sc']), 2))
                nc.leave_named_scope("b0", sc_, True)

                sc_ = nc.enter_named_scope("b1", True)[0]
                hch1, _, _ = load_hch(1, False)
                A1 = make_attn(1)
                g1 = {ob: block_gen(1, ob, hch1, p_w) for ob in ob_order}
                next(g1[HQ])                # prefetch b1 k,v weights
                next(g1[HQ + 1])
                # batch-0 attention tail rides batch-1's first blocks
                run_units((g1[HQ], 1),
                          (chain(onesh(A0[0]['fin']), A0[1]['cx'],
                                 A0[2]['sc']), 2))
                next(g1[0])
                run_units((g1[HQ + 1], 1),
                          (chain(onesh(A0[1]['fin']), A0[2]['cx'],
                                 A0[3]['sc']), 2))
                next(g1[1])
                run_units((g1[0], 1),
                          (chain(onesh(A0[2]['fin']), A0[3]['cx'],
                                 onesh(A0[3]['fin'])), 1))
                next(g1[2])
                drain(g1[1])
                next(g1[3])
                run_units((g1[2], 1), (A1[0]['sc'], 1))
                run_units((g1[3], 1),
                          (chain(A1[0]['cx'], A1[1]['sc']), 2))
                nc.leave_named_scope("b1", sc_, True)

            # ---------------- phase 2: b1 attn tail + o_proj ---------------
            with (
                tc.tile_pool(name="wo", bufs=1) as p_wo,
                tc.tile_pool(name="cx", bufs=1) as p_cx,
                tc.tile_pool(name="oo", bufs=5) as p_oo,
            ):
                WOB = ICH + 2

                def load_wo_grp(hgs, ics=None, dma=None):
                    wts = {}
                    if ics is None:
                        ics = range(ICH)
                    if dma is None:
                        dma = nc.scalar.dma_start
                    for ic in ics:
                        wo_t = p_wo.tile([128, GSZ * OH], BF16, tag="wo",
                                         name="wo", bufs=WOB)
                        dma(out=wo_t[:, :],
                            in_=wo[ic, :, hgs[0] * OH:(hgs[0] + GSZ) * OH])
                        for i, hg in enumerate(hgs):
                            wts[(hg, ic)] = wo_t[:, i * OH:(i + 1) * OH]
                    return wts

                # cx0 load can go early (a2a0 long done): one SWDGE DMA,
                # 1KB-contiguous per (partition, src core).
                cx_s = [p_cx.tile([128, ICH * TCB], BF16, tag=f"cx{b}",
                                  name=f"cx{b}") for b in range(B)]
                nc.gpsimd.dma_start(
                    out=cx_s[0][:, :].rearrange("p (j f) -> p j f", j=cores),
                    in_=a2a0_out[:, :].rearrange("(j p) f -> p j f", p=128))

                grp0_hgs = [i for i in range(GSZ)]
                grp0_wts = {}

                def wo_gen(ics):
                    def fn():
                        grp0_wts.update(load_wo_grp(
                            grp0_hgs, ics=ics, dma=nc.sync.dma_start))
                    return onesh(fn)

                sc_ = nc.enter_named_scope("tail", True)[0]
                flush_pending()          # finish b1-q3's rope
                A1[0]['fin']()
                run_units((chain(A1[1]['cx'], A1[2]['sc']), 2),
                          (wo_gen(range(0, 11)), 1))
                A1[1]['fin']()           # -> half0 AllToAll
                run_units((chain(A1[2]['cx'], A1[3]['sc']), 2),
                          (wo_gen(range(11, 22)), 1))
                A1[2]['fin']()
                run_units((A1[3]['cx'], 2), (wo_gen(range(22, 32)), 1))
                A1[3]['fin']()           # -> half1 AllToAll
                nc.leave_named_scope("tail", sc_, True)

                sc_ = nc.enter_named_scope("oproj", True)[0]
                # cx1 from the two half-collectives. On the sync queue (NOT
                # gpsimd): a load that waits on a collective would stall the
                # Pool FIFO and delay the other half's trigger.
                cxv = cx_s[1][:, :].rearrange("p (j h t) -> p j h t",
                                              j=cores, t=TCB)
                for pi in range(2):
                    nc.sync.dma_start(
                        out=cxv[:, :, pi * HH:(pi + 1) * HH, :],
                        in_=a2a1_out[pi][:, :].rearrange(
                            "(j p) (hh t) -> p j hh t", p=128, t=TCB))

                # batch 1 contracts the half0-delivered chunks first so its
                # o_proj can start before the half1 collective lands.
                ic_b1 = ([ic for ic in range(ICH) if ic % HQ < HH] +
                         [ic for ic in range(ICH) if ic % HQ >= HH])
                for grp in range(NHG // GSZ):
                    hgs = [grp * GSZ + i for i in range(GSZ)]
                    wts = grp0_wts if grp == 0 else load_wo_grp(hgs)
                    for b in range(B):
                        ics = ic_b1 if b == 1 else list(range(ICH))
                        psos = [ps_all.tile([TCB, OH], F32, tag="aux",
                                            name="pso", bufs=2)
                                for _ in range(GSZ)]
                        for i, ic in enumerate(ics):
                            lw = cx_s[b][:, ic * TCB:(ic + 1) * TCB]
                            for gi in range(GSZ):
                                nc.tensor.matmul(
                                    psos[gi][:, :],
                                    lhsT=lw,
                                    rhs=wts[(hgs[gi], ic)][:, :],
                                    start=(i == 0), stop=(i == ICH - 1))
                        for gi, hg in enumerate(hgs):
                            ot = p_oo.tile([TCB, OH], F32, tag="oout",
                                           name="oout")
                            nc.vector.tensor_copy(ot[:, :], psos[gi][:, :])
                            nc.sync.dma_start(
                                out=out[b * TCB:(b + 1) * TCB,
                                        hg * OH:(hg + 1) * OH],
                                in_=ot[:, :])
                nc.leave_named_scope("oproj", sc_, True)

    _pin_act_tables(nc.m.arch)
    nc.compile()
    return nc


def _pin_act_tables(arch):
    """Constrain the act-table chooser so Exp and Ln both resolve to the
    one set that contains them together (natural_log_exp_and_others).
    Otherwise the pass alternates exp_and_others <-> natural_log on every
    RMSNorm/softmax interleave, paying a ~1.3us table reload each time.
    Only the chooser's view is narrowed; the loaded set id and its table
    contents are unchanged, so results are bit-identical."""
    from concourse.hw_specs import get_activation_tables
    tabs = get_activation_tables(arch)
    both = [name for name, s in tabs.items()
            if any(f.name == 'Exp' for f in s)
            and any(f.name == 'Ln' for f in s)]
    if not both:
        return
    for name, s in tabs.items():
        if name not in both:
            for f in list(s):
                if f.name in ('Exp', 'Ln'):
                    s.discard(f)


def host_prep(inputs, B=2, S=1024, HID=4096, H=32, KV=8, D=128, eps=1e-6):
    """Shard + lay out the full inputs into per-core in_maps."""
    cores = N_CORES
    HQ = H // cores
    T = B * S
    HCH = HID // 128
    ICH = (H * D) // 128

    hs = np.ascontiguousarray(inputs["hidden_states"], dtype=np.float32)
    fc = np.asarray(inputs["freqs_cis"], dtype=np.float32)
    Wq = np.asarray(inputs["Wq"], dtype=np.float32)
    Wk = np.asarray(inputs["Wk"], dtype=np.float32)
    Wv = np.asarray(inputs["Wv"], dtype=np.float32)
    Wo = np.asarray(inputs["Wo"], dtype=np.float32)
    qnw = np.asarray(inputs["q_norm_w"], dtype=np.float32)
    knw = np.asarray(inputs["k_norm_w"], dtype=np.float32)

    # hidden^T chunks: hT[b, ch, p, s] = hs[b, s, ch*128+p]
    hT = np.ascontiguousarray(
        hs.transpose(0, 2, 1).reshape(B, HCH, 128, S)).astype(BF16_NP)

    cos, sin, nsin = fc[0], fc[1], fc[2]      # [S, D]
    cosT = np.ascontiguousarray(cos.T).astype(BF16_NP)    # [128, S]
    csinT = np.concatenate([nsin.T[0:64], sin.T[64:128]], axis=0)
    csinT = np.ascontiguousarray(csinT).astype(BF16_NP)
    qw_col = np.ascontiguousarray(qnw.reshape(128, 1))
    kw_col = np.ascontiguousarray(knw.reshape(128, 1))

    # Wo^T chunks: wo[ic, p, hid] = Wo[hid, ic*128+p]
    woT = np.ascontiguousarray(Wo.T.reshape(ICH, 128, HID)).astype(BF16_NP)

    def prep_w(Wm, nblocks):
        # [nblocks, p, ch*128] with w[ob, p, ch*128+j] = Wm[ob*128+j, ch*128+p]
        a = Wm.reshape(nblocks, 128, HCH, 128).transpose(0, 3, 2, 1)
        return np.ascontiguousarray(a.reshape(nblocks, 128, HCH * 128)) \
            .astype(BF16_NP)

    in_maps = []
    for c in range(cores):
        Wq_c = Wq[c * HQ * D:(c + 1) * HQ * D]
        Wk_c = Wk[c * D:(c + 1) * D]
        Wv_c = Wv[c * D:(c + 1) * D]
        in_maps.append({
            "hT": hT,
            "wq": prep_w(Wq_c, HQ),
            "wk": prep_w(Wk_c, 1)[0],
            "wv": prep_w(Wv_c, 1)[0],
            "wo": woT,
            "cosT": cosT,
            "csinT": csinT,
            "qw": qw_col,
            "kw": kw_col,
        })
    return in_maps


def gather_output(results, B=2, S=1024, HID=4096, **_):
    cores = N_CORES
    TCB = (B * S) // cores // B
    out = np.empty((B, S, HID), dtype=np.float32)
    for c in range(cores):
        o = results[c]["out"]
        for b in range(B):
            out[b, c * TCB:(c + 1) * TCB] = o[b * TCB:(b + 1) * TCB]
    return out


_NC_CACHE = {}


def kernel(**inputs) -> np.ndarray:
    cfg = FULL_CFG
    key = tuple(sorted(cfg.items()))
    if key not in _NC_CACHE:
        _NC_CACHE[key] = build_program(**cfg)
    nc = _NC_CACHE[key]
    in_maps = host_prep(inputs, **cfg)
    res = run_bass_kernel_spmd(nc, in_maps, core_ids=list(range(N_CORES)))
    return gather_output(res.results, **cfg)
